# revision 82
# baseline (speedup 1.0000x reference)
"""Linear attention kernel for 8 Trainium2 NeuronCores.

Sharding: core = 2*b + hg  (b in 0..3 batches, hg in 0..1 head-groups of 8 heads).
Fully data-parallel — no collectives; host sums the two head-group partials per
batch (f32) and adds the bias.

Per-core math (T=4096 tokens, CH=512 = 8 heads x 64, DIM=1024):
  Phase 1 (per 512-token block): k,v = x @ Wk, x @ Wv token-major; elu+1 on k;
    qT = (x @ Wq)^T c-major with elu+1 (persisted for phase 2);
    kvT += v^T k per head-pair (diagonal 64-blocks), z += ones^T k, in PSUM.
    The z/kv matmuls are deferred one token-chunk so the in-order PE never
    waits on the elu chain.
  Boundary: evict kvT/z; M = kvT^T @ W2 interleaved with the first den
    stages; Zb = block-diag expansion of z.
  Phase 2 (per 512-token block, software-pipelined 4 blocks deep so the
    in-order PE never waits on the r chain): den = Zb^T qT [8, 512];
    r = 1/(den+1e-6) computed in a [128,32] partition-blocked layout (DMA
    reshape both ways; 16x fewer elems per DVE lane than [8,512]);
    rbc = E^T r broadcast matmul; qsc = qT * rbc;
    y = qsc^T @ M, written bf16 (host accumulates in f32).

  A short burst of dummy matmuls at the start lifts the PE HAM clock gate
  (1.2 -> 2.4 GHz) while the first input DMAs are still in flight.
"""

import sys

sys.path.insert(0, "/opt/trn_rl_repo")

import numpy as np

import concourse.bass as bass
import concourse.mybir as mybir
import concourse.tile as tile
from concourse import bacc

F32 = mybir.dt.float32
BF16 = mybir.dt.bfloat16
F8 = mybir.dt.float8e4
AF = mybir.ActivationFunctionType
DR = mybir.MatmulPerfMode.DoubleRow

DIM = 1024      # model dim (contraction for projections)
CH = 512        # per-core channels (8 heads x 64)
P = 128

N_CORES = 8
B, T_FULL = 4, 4096

N_WARMUP = 24   # dummy matmuls to lift the HAM clock gate during input DMA
                # (too few leaves the PE clock stuck below 2.4 GHz for the
                # whole kernel: N_WARMUP=14 ran every matmul ~9% slower;
                # HAM grants full duty ~5us into the burst, so the last
                # few of 30 only delayed the first real matmul)

# k/q projections run in fp8e4 DoubleRow (2x PE rate).  W1 is scaled by
# WS host-side so its +-1/32 values clear the fp8 min-normal (2^-6); the
# elu descales via the activation scale.  v stays bf16: attention output
# is a weighted average of v, so v's quantization noise survives to the
# output undamped (k/q noise largely cancels in the num/den ratio).
WS = 16.0
WS_INV = 1.0 / WS
VC8 = 6         # of v's 8 contraction chunks, how many run in fp8 DR


def build_nc(T=T_FULL):
    NTB = T // 512          # 512-token blocks
    nc = bacc.Bacc(None, target_bir_lowering=False, debug=False)

    # fp8 operands for k/q (row-pair interleaved for DoubleRow); v runs
    # its first VC8 of 8 contraction chunks in fp8 too (noise scales with
    # sqrt of the fp8 fraction; 6/8 keeps rel err ~1.5e-2 < 2e-2) and the
    # last chunks in bf16, so only those x rows are uploaded in bf16.
    xT8 = nc.declare_dram_parameter("xT8", [4 * P, 2, T], F8, isOutput=False)
    xT = (nc.declare_dram_parameter("xT", [(8 - VC8) * P, T], BF16,
                                    isOutput=False) if VC8 < 8 else None)
    w18 = nc.declare_dram_parameter("w18", [4 * P, 2, 2 * CH], F8, isOutput=False)
    w1v8_d = nc.declare_dram_parameter("w1v8", [VC8 // 2 * P, 2, CH], F8,
                                       isOutput=False)
    w1v_d = (nc.declare_dram_parameter("w1v", [(8 - VC8) * P, CH], BF16,
                                       isOutput=False) if VC8 < 8 else None)
    w2 = nc.declare_dram_parameter("w2", [CH, DIM], BF16, isOutput=False)
    ec = nc.declare_dram_parameter("ec", [P, P], BF16, isOutput=False)
    km_d = nc.declare_dram_parameter("kmask", [P, 4 * P], BF16, isOutput=False)
    y = nc.declare_dram_parameter("y", [T, DIM], BF16, isOutput=True)

    with tile.TileContext(nc) as tc:
        with tc.tile_pool(name="persist", bufs=1) as pp:
            ones_col = pp.tile([P, 1], BF16, name="ones_col", tag="ones_col")
            nc.vector.memset(ones_col[:, :], 1.0)

            # k columns of W1 first: the first k-matmul group only needs
            # these + the first x block instead of all of W1.
            w1k, w1v8, w1v, w1q = [], [], [], []
            for ct in range(4):
                t_ = pp.tile([P, 2, CH], F8, name=f"w1k_{ct}", tag=f"w1k_{ct}")
                nc.sync.dma_start(
                    out=t_[:, :, :], in_=w18[ct * P:(ct + 1) * P, :, 0:CH]
                )
                w1k.append(t_)
                w1q.append(pp.tile([P, 2, CH], F8, name=f"w1q_{ct}", tag=f"w1q_{ct}"))
            for ct in range(VC8 // 2):
                w1v8.append(pp.tile([P, 2, CH], F8, name=f"w1v8_{ct}",
                                    tag=f"w1v8_{ct}"))
            for ct in range(8 - VC8):
                w1v.append(pp.tile([P, CH], BF16, name=f"w1v_{ct}", tag=f"w1v_{ct}"))

            qt = [
                [
                    pp.tile([P, 512], BF16, name=f"qt_{ib}_{j}", tag=f"qt_{ib}_{j}")
                    for j in range(4)
                ]
                for ib in range(NTB)
            ]

            kvt = pp.tile([P, 4 * P], BF16, name="kvt", tag="kvt")
            kmask = pp.tile([P, 4 * P], BF16, name="kmask", tag="kmask")
            zt = pp.tile([1, CH], BF16, name="zt", tag="zt")

            w2t, Ms = [], []
            for j in range(4):
                w2t.append(pp.tile([P, DIM], BF16, name=f"w2_{j}", tag=f"w2_{j}"))
                Ms.append(pp.tile([P, DIM], BF16, name=f"Ms_{j}", tag=f"Ms_{j}"))

            Zb = [
                pp.tile([P, 8], BF16, name=f"Zb_{j}", tag=f"Zb_{j}")
                for j in range(4)
            ]
            ec_sb = pp.tile([P, P], BF16, name="ec_sb", tag="ec_sb")

            phase1(nc, tc, pp, T, NTB, xT8, xT, w18, w1v8_d, w1v_d, w2, ec,
                   km_d, ec_sb, w1k, w1v8, w1v, w1q, qt, kvt, kmask, zt, w2t,
                   ones_col, Zb)
            phase2(nc, tc, pp, T, NTB, y, qt, kvt, w2t, Ms, Zb, ec_sb, zt,
                   ones_col)

    nc.compile()
    return nc


def premset_kv_z(nc, Zb):
    """Zero the Zb staging tiles up front (during the initial input DMA)
    so the end-of-phase-1 eviction is just the copies."""
    for j in range(4):
        nc.gpsimd.memset(Zb[j][:, :], 0.0)


def evict_kv_z(nc, kvps, zps, kvt, kmask, zt):
    """Evict the kv/z PSUM accumulators: one masked multiply frees the
    whole kvps bank (kmask zeroes the cross-head-pair blocks), and one
    scalar copy evicts z.  Zb staging happens at the phase-2 boundary
    (PE transpose + gpsimd copies) so nothing here blocks the M
    matmuls."""
    nc.vector.tensor_tensor(
        kvt[:, :], kvps[:, :], kmask[:, :], op=mybir.AluOpType.mult
    )
    nc.scalar.copy(zt[0:1, :], zps[0:1, :])


def phase1(nc, tc, pp, T, NTB, xT8, xT, w18, w1v8_d, w1v_d, w2, ec, km_d,
           ec_sb, w1k, w1v8, w1v, w1q, qt, kvt, kmask, zt, w2t, ones_col, Zb):
    with (
        tc.tile_pool(name="ph1_sb", bufs=3) as pa,
        tc.tile_pool(name="kv_sb", bufs=3) as kvp,
        tc.tile_pool(name="xload", bufs=24) as xp,
        tc.tile_pool(name="proj_ps", bufs=6, space="PSUM") as proj_ps,
        tc.tile_pool(name="hold_ps", bufs=1, space="PSUM") as hold_ps,
    ):
        # Warm the PE clock gate with dummy matmuls while input DMAs run.
        wu = pa.tile([P, 512], BF16, name="wu", tag="elu_e")
        nc.vector.memset(wu[:, :], 0.0)
        premset_kv_z(nc, Zb)
        wups = proj_ps.tile([P, 512], F32, name="wups", tag="proj")
        for i in range(N_WARMUP):
            nc.tensor.matmul(
                wups[:, :], wu[:, 0:P], wu[:, :],
                start=True, stop=True, skip_group_check=True,
            )

        # PSUM accumulators held across all of phase 1 (one bank each).
        kvps = hold_ps.tile([P, 4 * P], F32, name="kvps", tag="kvps")
        zps = hold_ps.tile([1, CH], F32, name="zps", tag="zps")
        nc.vector.memset(kvps[:, :], 0.0)

        pending = []            # deferred z/kv matmuls (closures)

        def flush_pending():
            while pending:
                pending.pop(0)()

        xt8s = {}

        def load_xt8(ib):
            lst = []
            for ct in range(4):
                t_ = xp.tile([P, 2, 512], F8, name=f"xt8_{ib}_{ct}", tag="xt8")
                nc.sync.dma_start(
                    out=t_[:, :, :],
                    in_=xT8[ct * P:(ct + 1) * P, :, ib * 512:(ib + 1) * 512],
                )
                lst.append(t_)
            xt8s[ib] = lst

        load_xt8(0)
        for ib in range(NTB):
            xt8 = xt8s.pop(ib)
            xt = []
            for ct in range(8 - VC8):
                t_ = xp.tile([P, 512], BF16, name=f"xt_{ib}_{ct}", tag="xt")
                nc.sync.dma_start(
                    out=t_[:, :], in_=xT[ct * P:(ct + 1) * P, ib * 512:(ib + 1) * 512]
                )
                xt.append(t_)
            if ib == 0:
                for ct in range(VC8 // 2):
                    nc.sync.dma_start(
                        out=w1v8[ct][:, :, :],
                        in_=w1v8_d[ct * P:(ct + 1) * P, :, :],
                    )
                for ct in range(8 - VC8):
                    nc.sync.dma_start(
                        out=w1v[ct][:, :], in_=w1v_d[ct * P:(ct + 1) * P, :]
                    )
                # ib1's fp8 x goes out ahead of w1q: its k-groups start
                # well before ib0's q-groups need the q weights.
                load_xt8(1)
                for ct in range(4):
                    nc.sync.dma_start(
                        out=w1q[ct][:, :, :],
                        in_=w18[ct * P:(ct + 1) * P, :, CH:2 * CH],
                    )
            elif ib + 1 < NTB:
                load_xt8(ib + 1)

            # k/v projections (token-major) per 128-token chunk, with the
            # z/kv matmuls of the previous chunk interleaved after each
            # vps group so they never wait on the elu chain.  For the first
            # block, all four k-groups run before any v-group so the PE
            # covers the in-flight w1v/w1q weight DMAs.
            k_sbs = {}

            # elu(x)+1 = min(exp(x), 1) + relu(x): exp reads PSUM directly
            # (no pre-min; overflow impossible, |x| < 6), the relu runs on
            # whichever of scalar/vector has slack, and a fused
            # (e min 1) + r scalar_tensor_tensor closes it in one fast
            # all-bf16 DVE op.
            def k_group(t):
                tok = slice(t * P, (t + 1) * P)
                kps = proj_ps.tile([P, 512], F32, name=f"kps_{ib}_{t}", tag="proj")
                for ct in range(4):
                    nc.tensor.matmul(
                        kps[:, :], xt8[ct][:, :, tok], w1k[ct][:, :, :],
                        start=(ct == 0), stop=(ct == 3), perf_mode=DR,
                    )
                ke = pa.tile([P, 512], BF16, name=f"ke_{ib}_{t}", tag="elu_e")
                kr = pa.tile([P, 512], BF16, name=f"kr_{ib}_{t}", tag="elu_r")
                k_sb = kvp.tile([P, 512], BF16, name=f"k_{ib}_{t}", tag="k_sb",
                                bufs=6)
                nc.scalar.activation(ke[:, :], kps[:, :], AF.Exp, scale=WS_INV)
                if ib == NTB - 1 and t == 3:
                    # last chunk gates the kv-stop -> eviction -> M chain:
                    # relu on vector, in parallel with scalar's exp
                    nc.vector.tensor_scalar(
                        kr[:, :], kps[:, :], 0.0, WS_INV,
                        mybir.AluOpType.max, mybir.AluOpType.mult,
                    )
                else:
                    nc.scalar.activation(kr[:, :], kps[:, :], AF.Relu,
                                         scale=WS_INV)
                nc.vector.scalar_tensor_tensor(
                    k_sb[:, :], ke[:, :], 1.0, kr[:, :],
                    mybir.AluOpType.min, mybir.AluOpType.add,
                )
                k_sbs[t] = k_sb

            def v_group(t):
                tok = slice(t * P, (t + 1) * P)
                vps = proj_ps.tile([P, 512], F32, name=f"vps_{ib}_{t}", tag="proj")
                nv8, nvb = VC8 // 2, 8 - VC8
                for ct in range(nv8):
                    nc.tensor.matmul(
                        vps[:, :], xt8[ct][:, :, tok], w1v8[ct][:, :, :],
                        start=(ct == 0), stop=(nvb == 0 and ct == nv8 - 1),
                        perf_mode=DR, skip_group_check=True,
                    )
                for ct in range(nvb):
                    nc.tensor.matmul(
                        vps[:, :], xt[ct][:, tok], w1v[ct][:, :],
                        start=False, stop=(ct == nvb - 1),
                        skip_group_check=True,
                    )
                v_sb = kvp.tile([P, 512], BF16, name=f"v_{ib}_{t}", tag="v_sb")
                nc.vector.tensor_copy(v_sb[:, :], vps[:, :])
                return v_sb

            def kv_loop():
                for t in range(4):
                    if ib != 0:
                        k_group(t)
                    v_sb = v_group(t)
                    k_sb = k_sbs[t]

                    flush_pending()

                    def defer(ib=ib, t=t, k_sb=k_sb, v_sb=v_sb):
                        first = (ib == 0 and t == 0)
                        last = (ib == NTB - 1 and t == 3)
                        # z += ones^T k   [1, 512]
                        nc.tensor.matmul(
                            zps[0:1, :], ones_col[:, :], k_sb[:, :],
                            start=first, stop=last, skip_group_check=True,
                        )
                        # kvT[j] += v_pair^T k_pair  [128,128] per head-pair;
                        # one accumulation region per j in the pre-zeroed bank.
                        for j in range(4):
                            csl = slice(j * P, (j + 1) * P)
                            nc.tensor.matmul(
                                kvps[:, csl], v_sb[:, csl], k_sb[:, csl],
                                start=False, stop=(last and j == 3),
                                skip_group_check=True,
                            )
                    pending.append(defer)

            # q projection (c-major) with elu+1, into persistent qt
            def q_group(j, flush=False):
                qps = proj_ps.tile([P, 512], F32, name=f"qps_{ib}_{j}", tag="proj")
                for ct in range(4):
                    nc.tensor.matmul(
                        qps[:, :],
                        w1q[ct][:, :, j * P:(j + 1) * P],
                        xt8[ct][:, :, :],
                        start=(ct == 0), stop=(ct == 3), perf_mode=DR,
                    )
                if flush:
                    flush_pending()
                qe = pa.tile([P, 512], BF16, name=f"qe_{ib}_{j}", tag="elu_e")
                qr = pa.tile([P, 512], BF16, name=f"qr_{ib}_{j}", tag="elu_r")
                nc.scalar.activation(qe[:, :], qps[:, :], AF.Exp, scale=WS_INV)
                nc.vector.tensor_scalar(
                    qr[:, :], qps[:, :], 0.0, WS_INV,
                    mybir.AluOpType.max, mybir.AluOpType.mult,
                )
                nc.vector.scalar_tensor_tensor(
                    qt[ib][j][:, :], qe[:, :], 1.0, qr[:, :],
                    mybir.AluOpType.min, mybir.AluOpType.add,
                )

            if ib == NTB - 1:
                # Last block: q j0-j2 run first so their elu tails (and the
                # PSUM-bank reads that block the boundary M matmuls via WAR)
                # drain during the k/v groups; only q j3 remains as PE cover
                # for the kv/z eviction.  Keeping the PE gap short here also
                # avoids tripping the HAM utilization throttle (it halves PE
                # duty for ~7us once triggered).
                for j in range(3):
                    q_group(j)
                kv_loop()
                flush_pending()
                evict_kv_z(nc, kvps, zps, kvt, kmask, zt)
                q_group(3)
            else:
                if ib == 0:
                    for t in range(4):
                        k_group(t)
                kv_loop()
                for j in range(4):
                    q_group(j, flush=(j == 0))

            if ib == 0:
                # stage phase-2 constants off the critical path
                for j in range(4):
                    nc.sync.dma_start(
                        out=w2t[j][:, :], in_=w2[j * P:(j + 1) * P, :]
                    )
                nc.sync.dma_start(out=ec_sb[:, :], in_=ec[:, :])
                nc.sync.dma_start(out=kmask[:, :], in_=km_d[:, :])

        flush_pending()


def phase2(nc, tc, pp, T, NTB, y, qt, kvt, w2t, Ms, Zb, ec_sb, zt, ones_col):
    Es = [ec_sb[32 * j:32 * j + 8, :] for j in range(4)]

    with (
        tc.tile_pool(name="ph2_sb", bufs=2) as pb,
        tc.tile_pool(name="qsc_pool", bufs=8) as qp,
    ):
        rTs = [None] * NTB

        def den_stage(ib, dps_pool, d_bufs=1):
            dps = dps_pool.tile([8, 512], F32, name=f"dps_{ib}", tag="d",
                                bufs=d_bufs)
            for j in range(4):
                nc.tensor.matmul(
                    dps[:, :], Zb[j][:, :], qt[ib][j][:, :],
                    start=(j == 0), stop=(j == 3),
                )
            den_sb = pb.tile([8, 512], F32, name=f"den_{ib}", tag="den_sb")
            nc.vector.tensor_scalar_add(den_sb[:, :], dps[:, :], 1e-6)
            # partition-blocked reshape: [8 heads, 512 tok] ->
            # [128 = head*16 + tok//32, 32 = tok%32]  (linearized DMA)
            den_rs = pb.tile([P, 32], F32, name=f"drs_{ib}", tag="den_rs")
            nc.sync.dma_start(out=den_rs[:, :], in_=den_sb[:, :])
            rr = pb.tile([P, 32], BF16, name=f"rr_{ib}", tag="rr")
            with nc.allow_low_precision(reason="r is O(1e-5); bf16 matches op dtype"):
                nc.vector.reciprocal(rr[:, :], den_rs[:, :])
            rT = pb.tile([P, 512], BF16, name=f"rT_{ib}", tag="rT", bufs=6)
            for g in range(4):
                nc.sync.dma_start(out=rT[32 * g:32 * g + 8, :], in_=rr[:, :])
            rTs[ib] = rT

        qscs = [None] * NTB

        def qsc_stage(ib, dps_pool, bc_bufs=4):
            qsc = []
            for j in range(4):
                bcp = dps_pool.tile([P, 512], F32, name=f"bcp_{ib}_{j}",
                                    tag="bc", bufs=bc_bufs)
                nc.tensor.matmul(
                    bcp[:, :], Es[j][:, :], rTs[ib][32 * j:32 * j + 8, :],
                    start=True, stop=True, tile_position=(32 * j, 0),
                )
                qs = qp.tile([P, 512], BF16, name=f"qsc_{ib}_{j}", tag="qsc")
                nc.vector.tensor_mul(qs[:, :], qt[ib][j][:, :], bcp[:, :])
                qsc.append(qs)
            qscs[ib] = qsc

        def y_stage(ib, yps_pool):
            qsc = qscs[ib]
            for t in range(4):
                tok = slice(t * P, (t + 1) * P)
                row = (ib * 4 + t) * P
                last = (ib == NTB - 1 and t == 3)
                y_sb = pb.tile([P, DIM], BF16, name=f"y_{ib}_{t}", tag="y_sb",
                               bufs=4)
                yps = [
                    yps_pool.tile([P, 512], F32, name=f"yps_{ib}_{t}_{h}",
                                  tag="y")
                    for h in range(2)
                ]
                # j-outer: one qsc[j] weight load feeds both h-halves
                for j in range(4):
                    for h in range(2):
                        nc.tensor.matmul(
                            yps[h][:, :], qsc[j][:, tok],
                            Ms[j][:, h * 512:(h + 1) * 512],
                            start=(j == 0), stop=(j == 3),
                        )
                for h in range(2):
                    hsl = slice(h * 512, (h + 1) * 512)
                    yp = yps[h]
                    if h == 0:
                        nc.vector.tensor_copy(y_sb[:, hsl], yp[:, :])
                        if last:
                            # store halves separately to shorten the tail
                            nc.sync.dma_start(
                                out=y[row:row + P, hsl], in_=y_sb[:, hsl]
                            )
                    else:
                        nc.scalar.copy(y_sb[:, hsl], yp[:, :])
                        if last:
                            nc.scalar.dma_start(
                                out=y[row:row + P, hsl], in_=y_sb[:, hsl]
                            )
                if not last:
                    # scalar is a HWDGE engine: issue the y store there to
                    # keep the sync queue free for the den->rT chain.
                    nc.scalar.dma_start(out=y[row:row + P, :], in_=y_sb[:, :])

        # Boundary: stage Zb (z transposed to partition-major via four
        # tiny PE transposes + same-partition gpsimd copies — no DMAs),
        # then interleave the M matmuls with the first four den stages
        # so the PE rides through the scalar-side M evictions.
        with tc.tile_pool(name="m_ps", bufs=4, space="PSUM") as mps_pool:
            # bf16 PSUM writes must be 4-byte aligned: use every other column
            zbT = mps_pool.tile([P, 8], BF16, name="zbT", tag="zbt", bufs=1)
            for j in range(4):
                nc.tensor.matmul(
                    zbT[:, 2 * j:2 * j + 1], zt[0:1, j * P:(j + 1) * P],
                    ones_col[0:1, 0:1], is_transpose=True,
                )
            for j in range(4):
                nc.vector.tensor_copy(
                    Zb[j][0:64, 2 * j:2 * j + 1], zbT[0:64, 2 * j:2 * j + 1]
                )
                nc.scalar.copy(
                    Zb[j][64:128, 2 * j + 1:2 * j + 2],
                    zbT[64:128, 2 * j:2 * j + 1],
                )
            for j in range(4):
                jsl = slice(j * P, (j + 1) * P)
                for h in range(2):
                    hsl = slice(h * 512, (h + 1) * 512)
                    mps = mps_pool.tile([P, 512], F32, name=f"mps_{j}_{h}",
                                        tag="m")
                    nc.tensor.matmul(
                        mps[:, :], kvt[:, jsl], w2t[j][:, hsl],
                        start=True, stop=True,
                    )
                    # scalar: the vector queue is busy with the phase-1
                    # tail and the den->r chain here
                    nc.scalar.copy(Ms[j][:, hsl], mps[:, :])
                den_stage(j, mps_pool)

        with (
            tc.tile_pool(name="d_ps", bufs=2, space="PSUM") as dps_pool,
            tc.tile_pool(name="y_ps", bufs=3, space="PSUM") as yps_pool,
        ):
            # qsc runs one iteration ahead of the y-stage that consumes it
            # so the y ldweights never wait on the vector queue.
            qsc_stage(0, dps_pool)
            for ib in range(4, NTB):
                qsc_stage(ib - 3, dps_pool)
                den_stage(ib, dps_pool)
                y_stage(ib - 4, yps_pool)
            for ib in range(NTB - 3, NTB):
                qsc_stage(ib, dps_pool)
                y_stage(ib - 1, yps_pool)
            y_stage(NTB - 1, yps_pool)


_NC_CACHE = {}


def _get_nc(T=T_FULL):
    if T not in _NC_CACHE:
        _NC_CACHE[T] = build_nc(T)
    return _NC_CACHE[T]


def _dr_interleave(a):
    """[D, F] -> [D//2, 2, F] with row d = ct*256 + i*128 + p at
    [ct*128 + p, i, :] — the row-pair layout DoubleRow matmuls consume."""
    nchunks, F = a.shape[0] // 256, a.shape[1]
    return np.ascontiguousarray(
        a.reshape(nchunks, 2, P, F).transpose(0, 2, 1, 3)
        .reshape(nchunks * P, 2, F)
    )


def make_in_maps(x, W_qkv, W_out, b_out):
    import ml_dtypes

    bf16 = ml_dtypes.bfloat16
    f8 = ml_dtypes.float8_e4m3
    x = np.asarray(x, dtype=np.float32)
    W_qkv = np.asarray(W_qkv, dtype=np.float32)
    W_out = np.asarray(W_out, dtype=np.float32)

    vsplit = VC8 * P  # first vsplit contraction rows of v run in fp8
    xTs, xT8s = [], []
    for b in range(B):
        xt = np.ascontiguousarray(x[b].T)
        if VC8 < 8:
            xTs.append(np.ascontiguousarray(xt[vsplit:].astype(bf16)))
        xT8s.append(_dr_interleave(xt.astype(f8)))
    w18s, w1v8s, w1vs, w2s = [], [], [], []
    for hg in range(2):
        cs = slice(hg * CH, (hg + 1) * CH)
        Wq = W_qkv[:, cs]
        Wk = W_qkv[:, DIM + hg * CH:DIM + (hg + 1) * CH]
        Wv = W_qkv[:, 2 * DIM + hg * CH:2 * DIM + (hg + 1) * CH]
        w18s.append(
            _dr_interleave(
                np.concatenate([Wk, Wq], axis=1).astype(np.float32) * WS
            ).astype(f8)
        )
        Wvs = Wv.astype(np.float32) * WS  # both v parts at 16x; W2 has 1/16
        w1v8s.append(_dr_interleave(Wvs[:vsplit]).astype(f8))
        if VC8 < 8:
            w1vs.append(np.ascontiguousarray(Wvs[vsplit:].astype(bf16)))
        w2s.append(np.ascontiguousarray((W_out[cs, :] * WS_INV).astype(bf16)))

    ecm = make_ec().astype(bf16)
    kmm = make_kmask().astype(bf16)
    in_maps = []
    for core in range(N_CORES):
        b, hg = core // 2, core % 2
        m = {
            "xT8": xT8s[b], "w18": w18s[hg], "w1v8": w1v8s[hg],
            "w2": w2s[hg], "ec": ecm, "kmask": kmm,
        }
        if VC8 < 8:
            m["xT"] = xTs[b]
            m["w1v"] = w1vs[hg]
        in_maps.append(m)
    return in_maps


def make_ec():
    """E selector staged per PE row-group: rows 32j..32j+8 hold E_j with
    E_j[h, p] = 1 iff head-of-partition-p-in-tile-j == h."""
    ecm = np.zeros((P, P), dtype=np.float32)
    for j in range(4):
        ecm[32 * j + 2 * j, 0:64] = 1.0
        ecm[32 * j + 2 * j + 1, 64:128] = 1.0
    return ecm


def make_kmask():
    """Block-diagonal selector for the one-op kvps eviction: 1 on the two
    per-head 64x64 diagonal blocks of each 128-col head-pair region."""
    m = np.zeros((P, 4 * P), dtype=np.float32)
    for j in range(4):
        m[0:64, j * P:j * P + 64] = 1.0
        m[64:128, j * P + 64:(j + 1) * P] = 1.0
    return m


def kernel(x, W_qkv, W_out, b_out):
    from concourse.bass_utils import run_bass_kernel_spmd

    nc = _get_nc(T_FULL)
    in_maps = make_in_maps(x, W_qkv, W_out, b_out)
    res = run_bass_kernel_spmd(nc, in_maps, core_ids=list(range(N_CORES))).results
    bo = np.asarray(b_out, dtype=np.float32)
    out = np.empty((B, T_FULL, DIM), dtype=np.float32)
    for b in range(B):
        out[b] = (res[2 * b]["y"].astype(np.float32)
                  + res[2 * b + 1]["y"].astype(np.float32) + bo)
    return out

